# revision 2
# baseline (speedup 1.0000x reference)
"""KNN column-imputation kernel (nn_ColProcessor) for 8 Trainium2 cores.

Strategy: shard the 4096 query rows across 8 cores (512 rows each, data
parallel per the row-independent structure). Each core streams its
[512, 16384] slice of dist_chunk through SBUF in [128, 16384] tiles:

  - ScalarE (ACT): negate in place (nd = -d), keeping the vector engine free
  - VectorE (DVE): native top-8 ops - `max` (8 largest per partition,
    descending) + `max_index` (their indices; duplicate-aware sequential
    matching == jax.lax.top_k's lowest-index-first tie-break)

The device returns the raw top-8 (smallest-distance) candidate indices per
query row. The host filters out non-donor fit rows (donor probability ~0.9,
so >= 5 of the top-8 raw candidates are donors for all but a handful of
rows) and averages the first K=5 donors. Rows with < 5 donor candidates
(or any NaN weirdness) fall back to an exact numpy replay of the reference
for just those rows. This keeps the device at the 2-DVE-pass floor with no
donor-mask pass over the 256MB stream.
"""

import sys

sys.path.insert(0, "/opt/trn_rl_repo")

import numpy as np

import concourse.bacc as bacc
import concourse.mybir as mybir
from concourse.tile import TileContext

N_Q, N_FIT, N_FEAT = 4096, 16384, 32
COL, K = 3, 5
BIG = 1.0e30
NAN_FILL = 1.0e10
N_CORES = 8
ROWS = N_Q // N_CORES  # 512 query rows per core
P = 128
N_TILES = ROWS // P  # 4

_EXEC_CACHE = {}


def _build(reps=1):
    nc = bacc.Bacc("TRN2", target_bir_lowering=False)
    d_in = nc.dram_tensor("d", [ROWS, N_FIT], mybir.dt.float32, kind="ExternalInput")
    i_out = nc.dram_tensor("idx", [ROWS, 8], mybir.dt.uint32, kind="ExternalOutput")

    with TileContext(nc) as tc:
        with (
            tc.tile_pool(name="work", bufs=2) as work,
            tc.tile_pool(name="small", bufs=4) as small,
        ):
            for _ in range(reps):
                for t in range(N_TILES):
                    dt = work.tile([P, N_FIT], mybir.dt.float32)
                    nc.sync.dma_start(out=dt, in_=d_in[t * P : (t + 1) * P, :])
                    nc.scalar.mul(out=dt, in_=dt, mul=-1.0)
                    v8 = small.tile([P, 8], mybir.dt.float32)
                    i8 = small.tile([P, 8], mybir.dt.uint32)
                    nc.vector.max(out=v8, in_=dt)
                    nc.vector.max_index(out=i8, in_max=v8, in_values=dt)
                    nc.sync.dma_start(out=i_out[t * P : (t + 1) * P, :], in_=i8)
    nc.finalize()
    return nc


def _get_exec(nc):
    """Cached jitted 8-core executor for a finalized Bass module.

    Mirrors bass2jax.run_bass_via_pjrt's multi-core path but memoizes the
    jitted function so repeated calls don't re-trace/re-compile, and accepts
    already-device-resident concat inputs.
    """
    key = id(nc)
    if key in _EXEC_CACHE:
        return _EXEC_CACHE[key]

    import jax
    from jax.sharding import Mesh, PartitionSpec
    from jax.experimental.shard_map import shard_map
    from concourse import bass2jax
    from concourse import mybir as _mybir

    bass2jax.install_neuronx_cc_hook()

    partition_name = nc.partition_id_tensor.name if nc.partition_id_tensor else None
    in_names, out_names, out_avals, zero_outs = [], [], [], []
    for alloc in nc.m.functions[0].allocations:
        if not isinstance(alloc, _mybir.MemoryLocationSet):
            continue
        name = alloc.memorylocations[0].name
        if alloc.kind == "ExternalInput":
            if name != partition_name:
                in_names.append(name)
        elif alloc.kind == "ExternalOutput":
            out_names.append(name)
            shape = tuple(alloc.tensor_shape)
            dtype = _mybir.dt.np(alloc.dtype)
            out_avals.append(jax.core.ShapedArray(shape, dtype))
            zero_outs.append(np.zeros(shape, dtype))
    n_params = len(in_names)
    n_outs = len(out_avals)
    all_in_names = list(in_names) + list(out_names)
    if partition_name is not None:
        all_in_names.append(partition_name)
    donate = tuple(range(n_params, n_params + n_outs))

    def _body(*args):
        operands = list(args)
        if partition_name is not None:
            operands.append(bass2jax.partition_id_tensor())
        outs = bass2jax._bass_exec_p.bind(
            *operands,
            out_avals=tuple(out_avals),
            in_names=tuple(all_in_names),
            out_names=tuple(out_names),
            lowering_input_output_aliases=(),
            sim_require_finite=True,
            sim_require_nnan=True,
            nc=nc,
        )
        return tuple(outs)

    devices = jax.devices()[:N_CORES]
    mesh = Mesh(np.asarray(devices), ("core",))
    in_specs = (PartitionSpec("core"),) * (n_params + n_outs)
    out_specs = (PartitionSpec("core"),) * n_outs
    jitted = jax.jit(
        shard_map(
            _body, mesh=mesh, in_specs=in_specs, out_specs=out_specs, check_rep=False
        ),
        donate_argnums=donate,
        keep_unused=True,
    )

    def run(concat_inputs):
        """concat_inputs: dict name -> (N_CORES*per_core_rows, ...) array."""
        args = [concat_inputs[n] for n in in_names]
        zeros = [
            np.zeros((N_CORES * z.shape[0], *z.shape[1:]), z.dtype) for z in zero_outs
        ]
        outs = jitted(*args, *zeros)
        return {n: outs[i] for i, n in enumerate(out_names)}

    _EXEC_CACHE[key] = run
    return run


_NC = None


def _device_top8(d):
    """d: [N_Q, N_FIT] f32 -> raw top-8 smallest-distance indices [N_Q, 8]."""
    global _NC
    if _NC is None:
        _NC = _build()
    run = _get_exec(_NC)
    out = run({"d": np.ascontiguousarray(d)})
    return np.asarray(out["idx"]).astype(np.int64)


def _exact_rows(d_rows, donor_ok, mask_fit_col, fitcol):
    """Exact numpy replay of the reference for a few rows: returns val[n]."""
    dm = np.where(
        donor_ok[None, :],
        np.where(np.isnan(d_rows), np.float32(NAN_FILL), d_rows),
        np.float32(BIG),
    )
    all_nan = np.all(np.isnan(d_rows) | ~donor_ok[None, :], axis=1)
    order = np.argsort(dm, axis=1, kind="stable")[:, :K]
    w = 1.0 - mask_fit_col[order].astype(np.float32)
    donors = fitcol[order]
    wsum = w.sum(axis=1)
    div = np.where(wsum == 0, np.float32(1.0), wsum)
    knn_val = (donors * w).sum(axis=1) / div
    obs = ~mask_fit_col
    msum = obs.sum(dtype=np.float32)
    col_sum = np.where(obs, fitcol, 0.0).sum(dtype=np.float32)
    col_mean = col_sum / (msum if msum > 0 else np.float32(1.0))
    return np.where(all_nan, col_mean, knn_val).astype(np.float32)


def kernel(
    X,
    dist_chunk,
    non_missing_fix_X,
    mask_fit_X,
    dist_idx_map,
    mask,
    row_missing_idx,
    _fit_X,
):
    X = np.asarray(X, dtype=np.float32)
    dist_chunk = np.asarray(dist_chunk, dtype=np.float32)
    non_missing_fix_X = np.asarray(non_missing_fix_X, dtype=bool)
    mask_fit_X = np.asarray(mask_fit_X, dtype=bool)
    mask = np.asarray(mask, dtype=bool)
    _fit_X = np.asarray(_fit_X, dtype=np.float32)
    rmi = np.asarray(row_missing_idx, dtype=np.int64)
    dmap = np.asarray(dist_idx_map, dtype=np.int64)

    gather_rows = dmap[rmi]
    if gather_rows.shape[0] == N_Q and np.array_equal(
        gather_rows, np.arange(N_Q, dtype=np.int64)
    ):
        d = dist_chunk
    else:
        d = np.ascontiguousarray(dist_chunk[gather_rows])
    assert d.shape == (N_Q, N_FIT)

    idx8 = _device_top8(d)

    donor_ok = non_missing_fix_X[:, COL]
    fitcol = _fit_X[:, COL]
    mask_fit_col = mask_fit_X[:, COL]

    donor8 = donor_ok[idx8]
    cnt = donor8.sum(axis=1)
    good = cnt >= K

    # first K donor slots, preserving (value, index) candidate order
    key = (~donor8) * 8 + np.arange(8)[None, :]
    sel = np.argsort(key, axis=1, kind="stable")[:, :K]
    idx5 = np.take_along_axis(idx8, sel, axis=1)

    w = 1.0 - mask_fit_col[idx5].astype(np.float32)
    donors = fitcol[idx5]
    wsum = w.sum(axis=1)
    div = np.where(wsum == 0, np.float32(1.0), wsum)
    val = (donors * w).sum(axis=1) / div

    if not good.all():
        bad = np.flatnonzero(~good)
        val[bad] = _exact_rows(d[bad], donor_ok, mask_fit_col, fitcol)

    col_mask = mask[rmi, COL]
    new_col = np.where(col_mask, val, X[rmi, COL]).astype(np.float32)
    out = X.copy()
    out[rmi, COL] = new_col
    return out
